# revision 16
# baseline (speedup 1.0000x reference)
# Trainium2 Bass kernel for nn_DNA_Module_10780367912992 (scrambled-unfold
# sigmoid neighborhood attention, 4 cross branches + fuse/MLP/LN).
#
# Sharding: spatial row-sharding across 8 cores. Core c owns image rows
# [8c, 8c+8) (512 tokens). K/V are computed on a 26-row halo slice per core;
# everything else is per-token. No collectives; host gathers row slices.
#
# On-device layout is channels-on-partitions throughout ("X^T" layout):
#  - linear layers:  Y^T = W @ X^T via PE matmuls (lhsT = W^T, k-chunked)
#  - bias+zero-halo handled by a host-built validity row appended to X^T
#  - NA: for each of the 49 offsets p, the scrambled einsum decomposes into
#      prodK[(h,c),n] = K[(h,c), n+off_p] * Q[(h,(17c+p)%32), n]
#      scores[(h,p'),n] += prodK where p' = (49c+p)//32        (PE scatter)
#      P = sigmoid(scores + rpb)                               (ACT)
#      prodV[(h,k),n] = V[(h,k), n+off_p] * P[(h,(49k+p)//32), n]
#      out[(h,c'),n] += prodV where c' = (49k+p)%32            (PE scatter)
#    The Q-permutation / P-expansion are PE matmuls with 0/1 matrices; the
#    shifted K/V reads are zero-copy APs into a zero-padded [26x82] image.

import numpy as np

import concourse.bass as bass
import concourse.tile as tile
from concourse import bacc, mybir
from concourse.bass_utils import run_bass_kernel_spmd

F32 = mybir.dt.float32
BF16 = mybir.dt.bfloat16
AF = mybir.ActivationFunctionType
OP = mybir.AluOpType

B, HS, WS = 1, 64, 64
E, H, HD = 256, 8, 32
KS, DIL = 7, 3
DS = DC = 256
N = HS * WS
SCALE = HD ** 0.5
NCORES = 8
ROWS = HS // NCORES            # 8 own rows per core
NOWN = ROWS * WS               # 512
HALO = DIL * (KS - 1) // 2     # 9
HROWS = ROWS + 2 * HALO        # 26
NH = HROWS * WS                # 1664
PADX = HALO
WP = WS + 2 * PADX             # 82
IMGF = HROWS * WP              # 2132
OWN0 = HALO * WP + PADX        # own-pixel (0,0) offset in padded image: 747
NP = KS * KS                   # 49
OFFS = [((i - 3) * DIL, (j - 3) * DIL) for i in range(KS) for j in range(KS)]

import ml_dtypes
# stream dtype for heavy tensors; f32 kept for PSUM, LN internals, residuals
DT = BF16
NDT = np.float32
NBF = ml_dtypes.bfloat16


# ---------------------------------------------------------------- host prep

def _sel_matrices():
    ks = np.arange(HD)
    qperm = np.zeros((128, NP * 128), NDT)
    ksel = np.zeros((64, NP * 98), NDT)
    ssel = np.zeros((98, NP * 64), NDT)
    csel = np.zeros((64, NP * 64), NDT)
    for p in range(NP):
        for h in range(4):
            for c in range(HD):
                qperm[h * 32 + (17 * c + p) % 32, p * 128 + h * 32 + c] = 1.0
        for hh in range(2):
            for c in range(HD):
                ksel[hh * 32 + c, p * 98 + hh * NP + (49 * c + p) // 32] = 1.0
                ssel[hh * NP + (49 * c + p) // 32, p * 64 + hh * 32 + c] = 1.0
                csel[hh * 32 + c, p * 64 + hh * 32 + (49 * c + p) % 32] = 1.0
    return qperm, ksel, ssel, csel


def _np(x):
    return np.asarray(x, dtype=np.float32)


def _prep_weights(p):
    """Host-side parameter repack. Returns dict of DRAM input arrays (shared
    across cores)."""
    d = {}

    def lin_T_ones(l, scale=1.0):
        w = _np(l["w"]) * scale        # [dout, din]
        b = _np(l["b"]) * scale        # [dout]
        return np.concatenate([w.T, b[None, :]], 0).astype(NDT)  # [din+1,dout]

    for br, (qn, kn, vn, on) in enumerate(
        [("q_ss", "k_ss", "v_ss", "out_ss"), ("q_sc", "k_sc", "v_sc", "out_sc"),
         ("q_cc", "k_cc", "v_cc", "out_cc"), ("q_cs", "k_cs", "v_cs", "out_cs")]
    ):
        d[f"wq{br}"] = lin_T_ones(p[qn], 1.0 / SCALE).astype(NBF)  # [257,256]
        d[f"wk{br}"] = lin_T_ones(p[kn]).astype(NBF)
        d[f"wv{br}"] = lin_T_ones(p[vn]).astype(NBF)
        d[f"wo{br}"] = _np(p[on]["w"]).T.astype(NBF)    # [256, 256]
        d[f"bo{br}"] = _np(p[on]["b"])[:, None].astype(NDT)  # [256, 1]

    for path, fuse, m1, m2, ln in [
        ("s", "fuse_sam", "mlp_sam_1", "mlp_sam_2", "ln_sam"),
        ("c", "fuse_cnn", "mlp_cnn_1", "mlp_cnn_2", "ln_cnn"),
    ]:
        fw = _np(p[fuse]["w"])                           # [256, 512]
        d[f"wfa{path}"] = fw[:, :256].T.astype(NBF)
        d[f"wfb{path}"] = fw[:, 256:].T.astype(NBF)
        d[f"bf{path}"] = _np(p[fuse]["b"])[:, None].astype(NDT)
        d[f"wm1{path}"] = _np(p[m1]["w"]).T.astype(NBF)  # [256, 1024]
        bm1 = _np(p[m1]["b"])                            # [1024]
        d[f"bm1{path}"] = bm1.reshape(8, 128).T.astype(NDT)  # [128, 8] col/chunk
        d[f"wm2{path}"] = _np(p[m2]["w"]).T.astype(NBF)  # [1024, 256]
        d[f"bm2{path}"] = _np(p[m2]["b"])[:, None].astype(NDT)
        d[f"gln{path}"] = _np(p[ln]["g"])[:, None].astype(NDT)
        d[f"bln{path}"] = _np(p[ln]["b"])[:, None].astype(NDT)

    d["gnms"] = _np(p["norm_sam"]["g"])[:, None].astype(NDT)
    d["bnms"] = _np(p["norm_sam"]["b"])[:, None].astype(NDT)
    d["gnmc"] = _np(p["norm_cnn"]["g"])[:, None].astype(NDT)
    d["bnmc"] = _np(p["norm_cnn"]["b"])[:, None].astype(NDT)

    rpb = _np(p["rpb"]).reshape(H, NP)                   # [8, 49]
    for pp in range(4):
        d[f"rpb{pp}"] = np.concatenate(
            [rpb[2 * pp], rpb[2 * pp + 1]], 0)[:, None].astype(NDT)  # [98,1]

    qperm, ksel, ssel, csel = _sel_matrices()
    d["qperm"], d["ksel"] = qperm.astype(NBF), ksel.astype(NBF)
    d["ssel"], d["csel"] = ssel.astype(NBF), csel.astype(NBF)
    return d


# ---------------------------------------------------------- device program

def _ap3(t_ap, off, d1, d2):
    """[128, (d1 rows, d2 cols)] view of a 2D tile with row stride built in."""
    return bass.AP(tensor=t_ap.tensor, offset=t_ap.offset + off,
                   ap=[t_ap.ap[0], list(d1), list(d2)])


def _build(nc):
    from contextlib import ExitStack
    ctx = ExitStack()
    tc = ctx.enter_context(tile.TileContext(nc))

    din = {}
    def dram_in(name, shape, dt=F32):
        din[name] = nc.dram_tensor(name, shape, dt, kind="ExternalInput")
        return din[name]

    # inputs
    dram_in("ximg", [E, NH], BF16); dram_in("xcnn", [E, NH], BF16)
    dram_in("vrow", [1, NH], BF16)
    dram_in("rimg", [E, NOWN]); dram_in("rcnn", [E, NOWN])
    for br in range(4):
        dram_in(f"wq{br}", [E + 1, E], BF16); dram_in(f"wk{br}", [E + 1, E], BF16)
        dram_in(f"wv{br}", [E + 1, E], BF16); dram_in(f"wo{br}", [E, E], BF16)
        dram_in(f"bo{br}", [E, 1])
    for path in "sc":
        dram_in(f"wfa{path}", [E, E], BF16); dram_in(f"wfb{path}", [E, E], BF16)
        dram_in(f"bf{path}", [E, 1])
        dram_in(f"wm1{path}", [E, 4 * E], BF16); dram_in(f"bm1{path}", [128, 8])
        dram_in(f"wm2{path}", [4 * E, E], BF16); dram_in(f"bm2{path}", [E, 1])
        dram_in(f"gln{path}", [E, 1]); dram_in(f"bln{path}", [E, 1])
    dram_in("gnms", [E, 1]); dram_in("bnms", [E, 1])
    dram_in("gnmc", [E, 1]); dram_in("bnmc", [E, 1])
    for pp in range(4):
        dram_in(f"rpb{pp}", [98, 1])
    dram_in("qperm", [128, NP * 128], BF16); dram_in("ksel", [64, NP * 98], BF16)
    dram_in("ssel", [98, NP * 64], BF16); dram_in("csel", [64, NP * 64], BF16)

    osam = nc.dram_tensor("osam", [E, NOWN], F32, kind="ExternalOutput")
    ocnn = nc.dram_tensor("ocnn", [E, NOWN], F32, kind="ExternalOutput")

    consts = ctx.enter_context(tc.tile_pool(name="consts", bufs=1))
    imgs = ctx.enter_context(tc.tile_pool(name="imgs", bufs=1))
    work = ctx.enter_context(tc.tile_pool(name="work", bufs=1))
    pp = ctx.enter_context(tc.tile_pool(name="pp", bufs=3))
    acts = ctx.enter_context(tc.tile_pool(name="acts", bufs=1))
    ps_a = ctx.enter_context(tc.tile_pool(name="ps_a", bufs=2, space="PSUM"))
    ps_p = ctx.enter_context(tc.tile_pool(name="ps_p", bufs=1, space="PSUM"))
    ps_sc = ctx.enter_context(tc.tile_pool(name="ps_sc", bufs=1, space="PSUM"))
    ps_v = ctx.enter_context(tc.tile_pool(name="ps_v", bufs=1, space="PSUM"))
    ps_ln = ctx.enter_context(tc.tile_pool(name="ps_ln", bufs=1, space="PSUM"))

    def load(name, shape=None, dt=None):
        dt = dt or din[name].dtype
        t = consts.tile(shape or list(din[name].shape), dt, tag=name, name=name)
        nc.sync.dma_start(out=t, in_=din[name][:])
        return t

    def load2(name, dt=None):
        """Load a >=128-row DRAM tensor as a list of [128, cols] tiles."""
        dt = dt or din[name].dtype
        rows, cols = din[name].shape
        out = []
        for i in range(0, rows, 128):
            t = consts.tile([min(128, rows - i), cols], dt, tag=f"{name}_{i}", name=f"{name}_{i}")
            nc.sync.dma_start(out=t, in_=din[name][i:i + min(128, rows - i), :])
            out.append(t)
        return out

    # resident constants
    sb = {}
    for br in range(4):
        for w in ("wq", "wk", "wv"):
            full = din[f"{w}{br}"]
            a = consts.tile([128, E], DT, tag=f"{w}{br}a", name=f"{w}{br}a")
            b = consts.tile([128, E], DT, tag=f"{w}{br}b", name=f"{w}{br}b")
            r = consts.tile([1, E], DT, tag=f"{w}{br}r", name=f"{w}{br}r")
            nc.sync.dma_start(out=a, in_=full[0:128, :])
            nc.sync.dma_start(out=b, in_=full[128:256, :])
            nc.sync.dma_start(out=r, in_=full[256:257, :])
            sb[f"{w}{br}"] = (a, b, r)
        sb[f"wo{br}"] = load2(f"wo{br}")
        sb[f"bo{br}"] = load2(f"bo{br}")
    for path in "sc":
        for nm in (f"wfa{path}", f"wfb{path}", f"bf{path}", f"wm1{path}",
                   f"wm2{path}", f"bm2{path}", f"gln{path}", f"bln{path}"):
            sb[nm] = load2(nm)
        sb[f"bm1{path}"] = load(f"bm1{path}")
    for nm in ("gnms", "bnms", "gnmc", "bnmc"):
        sb[nm] = load2(nm)
    for nm in ("qperm", "ssel", "rpb0", "rpb1", "rpb2", "rpb3"):
        sb[nm] = load(nm)
    for nm in ("ksel", "csel"):
        cols = din[nm].shape[1]
        t = consts.tile([128, cols], BF16, tag=nm, name=nm)
        nc.sync.dma_start(out=t[0:64, :], in_=din[nm][:])
        nc.sync.dma_start(out=t[64:128, :], in_=din[nm][:])
        sb[nm] = t

    ones128 = consts.tile([128, 1], F32, tag="ones128", name="ones128")
    nc.vector.memset(ones128, 1.0)
    eps1 = consts.tile([1, 1], F32, tag="eps1", name="eps1")
    nc.vector.memset(eps1, 1e-5)

    # activations: x^T tiles (k-chunks) + validity row
    xi_a = acts.tile([128, NH], DT, tag="xi_a", name="xi_a")
    xi_b = acts.tile([128, NH], DT, tag="xi_b", name="xi_b")
    xc_a = acts.tile([128, NH], DT, tag="xc_a", name="xc_a")
    xc_b = acts.tile([128, NH], DT, tag="xc_b", name="xc_b")
    vrow = acts.tile([1, NH], DT, tag="vrow", name="vrow")
    nc.sync.dma_start(out=xi_a, in_=din["ximg"][0:128, :])
    nc.sync.dma_start(out=xi_b, in_=din["ximg"][128:256, :])
    nc.sync.dma_start(out=xc_a, in_=din["xcnn"][0:128, :])
    nc.sync.dma_start(out=xc_b, in_=din["xcnn"][128:256, :])
    nc.sync.dma_start(out=vrow, in_=din["vrow"][:])
    rimg = [acts.tile([128, NOWN], F32, tag=f"rimg{i}", name=f"rimg{i}") for i in range(2)]
    rcnn = [acts.tile([128, NOWN], F32, tag=f"rcnn{i}", name=f"rcnn{i}") for i in range(2)]
    for i in range(2):
        nc.sync.dma_start(out=rimg[i], in_=din["rimg"][i * 128:(i + 1) * 128, :])
        nc.sync.dma_start(out=rcnn[i], in_=din["rcnn"][i * 128:(i + 1) * 128, :])

    OC0 = HALO * WS                    # own-column start in halo token space

    def proj(wkey, xab, cols, psum, m0, start=True, stop=True):
        """psum[128, len] += W[m0:m0+128,:] @ x[:, cols] with bias via vrow."""
        a, b, r = sb[wkey]
        xa, xb = xab
        c0, c1 = cols
        nc.tensor.matmul(psum, a[:, m0:m0 + 128], xa[:, c0:c1], start=start, stop=False)
        nc.tensor.matmul(psum, b[:, m0:m0 + 128], xb[:, c0:c1], start=False, stop=False)
        nc.tensor.matmul(psum, r[:, m0:m0 + 128], vrow[:, c0:c1], start=False, stop=stop)

    # ---------------- per-branch compute ----------------
    branch_src = [  # (q-source, kv-source)
        ((xi_a, xi_b), (xi_a, xi_b)), ((xi_a, xi_b), (xc_a, xc_b)),
        ((xc_a, xc_b), (xc_a, xc_b)), ((xc_a, xc_b), (xi_a, xi_b)),
    ]
    outT = {}   # branch -> [2 tiles [128, NOWN]] attention+out-proj result

    ROWCH = [(0, 7), (7, 14), (14, 20), (20, 26)]  # halo row chunks

    for br in range(4):
        qsrc, kvsrc = branch_src[br]

        # K/V padded images per head-group: img[(4h x 32c), 26*82]
        kimg = [imgs.tile([128, IMGF], DT, tag=f"kimg{g}", name=f"kimg{g}") for g in range(2)]
        vimg = [imgs.tile([128, IMGF], DT, tag=f"vimg{g}", name=f"vimg{g}") for g in range(2)]
        for g in range(2):
            nc.vector.memset(kimg[g], 0.0)
            nc.vector.memset(vimg[g], 0.0)
        for side, wname, dst in (("k", "wk", kimg), ("v", "wv", vimg)):
            for g in range(2):
                for (r0, r1) in ROWCH:
                    nr = r1 - r0
                    fd = nr * WS
                    psum = ps_p.tile([128, 512], F32, tag="ps_proj", name="ps_proj")
                    proj(f"{wname}{br}", kvsrc, (r0 * WS, r1 * WS),
                         psum[:, 0:fd], g * 128)
                    pap = psum[:]
                    src = bass.AP(tensor=pap.tensor, offset=pap.offset,
                                  ap=[pap.ap[0], [WS, nr], [1, WS]])
                    dstap = _ap3(dst[g][:], r0 * WP + PADX, [WP, nr], [1, WS])
                    nc.vector.tensor_copy(dstap, src)

        # Q per group [128 = 4h x 32c, NOWN]
        q4 = [work.tile([128, NOWN], DT, tag="q4", name="q4") for _ in range(2)]
        for g in range(2):
            psum = ps_p.tile([128, 512], F32, tag="ps_proj", name="ps_proj")
            proj(f"wq{br}", qsrc, (OC0, OC0 + NOWN), psum, g * 128)
            nc.scalar.activation(q4[g][:], psum[:], AF.Identity)

        attn = [work.tile([128, NOWN], DT, tag=f"attn{g}", name=f"attn{g}") for g in range(2)]

        for g in range(2):
            # ---- K loop: scores ----
            sc01 = ps_sc.tile([98, NOWN], F32, tag="sc01", name="sc01")
            sc23 = ps_sc.tile([98, NOWN], F32, tag="sc23", name="sc23")
            for p in range(NP):
                di, dj = OFFS[p]
                off = di * WP + dj
                qp = ps_a.tile([128, NOWN], F32, tag="qp", name="qp")
                nc.tensor.matmul(qp, sb["qperm"][:, p * 128:(p + 1) * 128],
                                 q4[g], start=True, stop=True)
                prod = pp.tile([128, NOWN], DT, tag="prod", name="prod")
                in0 = _ap3(kimg[g][:], OWN0 + off, [WP, ROWS], [1, WS])
                in1 = _ap3(qp[:], 0, [WS, ROWS], [1, WS])
                o3 = _ap3(prod[:], 0, [WS, ROWS], [1, WS])
                nc.vector.tensor_tensor(out=o3, in0=in0, in1=in1, op=OP.mult)
                nc.tensor.matmul(sc01, sb["ksel"][0:64, p * 98:(p + 1) * 98],
                                 prod[0:64, :], start=(p == 0),
                                 stop=(p == NP - 1))
                nc.tensor.matmul(sc23, sb["ksel"][64:128, p * 98:(p + 1) * 98],
                                 prod[64:128, :], start=(p == 0),
                                 stop=(p == NP - 1))
            # ---- sigmoid(scores + rpb) ----
            p01 = work.tile([98, NOWN], DT, tag="p01", name="p01")
            p23 = work.tile([98, NOWN], DT, tag="p23", name="p23")
            nc.scalar.activation(p01[:], sc01[:], AF.Sigmoid,
                                 bias=sb[f"rpb{2 * g}"][:], scale=1.0)
            nc.scalar.activation(p23[:], sc23[:], AF.Sigmoid,
                                 bias=sb[f"rpb{2 * g + 1}"][:], scale=1.0)
            # ---- V loop ----
            vout0 = ps_v.tile([64, NOWN], F32, tag="vout0", name="vout0")
            vout1 = ps_v.tile([64, NOWN], F32, tag="vout1", name="vout1")
            for p in range(NP):
                di, dj = OFFS[p]
                off = di * WP + dj
                prep = ps_a.tile([128, NOWN], F32, tag="qp", name="prep")
                ssl = sb["ssel"][:, p * 64:(p + 1) * 64]
                nc.tensor.matmul(prep[0:64, :], ssl, p01, start=True, stop=True)
                nc.tensor.matmul(prep[64:128, :], ssl, p23, start=True, stop=True)
                prodv = pp.tile([128, NOWN], DT, tag="prodv", name="prodv")
                in0 = _ap3(vimg[g][:], OWN0 + off, [WP, ROWS], [1, WS])
                in1 = _ap3(prep[:], 0, [WS, ROWS], [1, WS])
                o3 = _ap3(prodv[:], 0, [WS, ROWS], [1, WS])
                nc.vector.tensor_tensor(out=o3, in0=in0, in1=in1, op=OP.mult)
                nc.tensor.matmul(vout0[:], sb["csel"][0:64, p * 64:(p + 1) * 64],
                                 prodv[0:64, :], start=(p == 0), stop=(p == NP - 1))
                nc.tensor.matmul(vout1[:], sb["csel"][64:128, p * 64:(p + 1) * 64],
                                 prodv[64:128, :], start=(p == 0), stop=(p == NP - 1))
            nc.scalar.activation(attn[g][0:64, :], vout0[:], AF.Identity)
            nc.scalar.activation(attn[g][64:128, :], vout1[:], AF.Identity)

        # ---- out projection [256, NOWN] ----
        oT = [work.tile([128, NOWN], DT, tag=f"oT{br}_{g}", name=f"oT{br}_{g}") for g in range(2)]
        wo, bo = sb[f"wo{br}"], sb[f"bo{br}"]
        for m in range(2):
            psum = ps_p.tile([128, NOWN], F32, tag="ps_proj", name="ps_proj")
            nc.tensor.matmul(psum, wo[0][:, m * 128:(m + 1) * 128], attn[0],
                             start=True, stop=False)
            nc.tensor.matmul(psum, wo[1][:, m * 128:(m + 1) * 128], attn[1],
                             start=False, stop=True)
            nc.scalar.activation(oT[m][:], psum[:], AF.Identity,
                                 bias=bo[m][:], scale=1.0)
        outT[br] = oT

    # ---------------- fuse + MLP + LN per path ----------------
    def layer_norm(xin2, g_ap, b_ap, out2):
        """LN over channel dim (256 = 2 partition chunks) of [2][128, NOWN]."""
        mu = ps_ln.tile([1, NOWN], F32, tag="ln_ps", name="ln_mu")
        nc.tensor.matmul(mu, ones128, xin2[0], start=True, stop=False)
        nc.tensor.matmul(mu, ones128, xin2[1], start=False, stop=True)
        mu_s = work.tile([1, NOWN], F32, tag="ln_mu_s", name="ln_mu_s")
        nc.scalar.activation(mu_s[:], mu[:], AF.Identity, scale=1.0 / E)
        mu_b = work.tile([128, NOWN], F32, tag="ln_mu_b", name="ln_mu_b")
        nc.gpsimd.partition_broadcast(mu_b[:], mu_s[:])
        xc = [work.tile([128, NOWN], F32, tag=f"ln_xc{i}", name=f"ln_xc{i}") for i in range(2)]
        sq = [work.tile([128, NOWN], F32, tag=f"ln_sq{i}", name=f"ln_sq{i}") for i in range(2)]
        for i in range(2):
            nc.vector.tensor_tensor(out=xc[i][:], in0=xin2[i][:], in1=mu_b[:],
                                    op=OP.subtract)
            nc.scalar.activation(sq[i][:], xc[i][:], AF.Square)
        var = ps_ln.tile([1, NOWN], F32, tag="ln_ps", name="ln_var")
        nc.tensor.matmul(var, ones128, sq[0], start=True, stop=False)
        nc.tensor.matmul(var, ones128, sq[1], start=False, stop=True)
        sd = work.tile([1, NOWN], F32, tag="ln_sd", name="ln_sd")
        nc.scalar.activation(sd[:], var[:], AF.Sqrt, bias=eps1[:], scale=1.0 / E)
        rs = work.tile([1, NOWN], F32, tag="ln_rs", name="ln_rs")
        nc.vector.reciprocal(rs[:], sd[:])
        rs_b = work.tile([128, NOWN], F32, tag="ln_rs_b", name="ln_rs_b")
        nc.gpsimd.partition_broadcast(rs_b[:], rs[:])
        for i in range(2):
            xn = work.tile([128, NOWN], F32, tag=f"ln_xn{i}", name=f"ln_xn{i}")
            nc.vector.tensor_tensor(out=xn[:], in0=xc[i][:], in1=rs_b[:],
                                    op=OP.mult)
            nc.scalar.activation(out2[i][:], xn[:], AF.Identity,
                                 scale=g_ap[i][:], bias=b_ap[i][:])

    for path, (ba, bb), res_ab, gnm, bnm, odram in (
        ("s", (0, 1), rimg, "gnms", "bnms", osam),
        ("c", (2, 3), rcnn, "gnmc", "bnmc", ocnn),
    ):
        a0, a1 = outT[ba]
        b0, b1 = outT[bb]
        wfa, wfb = sb[f"wfa{path}"], sb[f"wfb{path}"]
        fuse = [work.tile([128, NOWN], DT, tag=f"fuse{i}", name=f"fuse{i}") for i in range(2)]
        for m in range(2):
            ms = slice(m * 128, (m + 1) * 128)
            psum = ps_p.tile([128, NOWN], F32, tag="ps_proj", name="ps_proj")
            nc.tensor.matmul(psum, wfa[0][:, ms], a0, start=True, stop=False)
            nc.tensor.matmul(psum, wfa[1][:, ms], a1, start=False, stop=False)
            nc.tensor.matmul(psum, wfb[0][:, ms], b0, start=False, stop=False)
            nc.tensor.matmul(psum, wfb[1][:, ms], b1, start=False, stop=True)
            nc.scalar.activation(fuse[m][:], psum[:], AF.Identity,
                                 bias=sb[f"bf{path}"][m][:], scale=1.0)
        wm1, bm1 = sb[f"wm1{path}"], sb[f"bm1{path}"]
        h1 = [work.tile([128, NOWN], DT, tag=f"h1_{j}", name=f"h1_{j}") for j in range(8)]
        for j in range(8):
            ms = slice(j * 128, (j + 1) * 128)
            psum = ps_p.tile([128, NOWN], F32, tag="ps_proj", name="ps_proj")
            nc.tensor.matmul(psum, wm1[0][:, ms], fuse[0], start=True, stop=False)
            nc.tensor.matmul(psum, wm1[1][:, ms], fuse[1], start=False, stop=True)
            nc.scalar.activation(h1[j][:], psum[:], AF.Gelu,
                                 bias=bm1[:, j:j + 1], scale=1.0)
        wm2, bm2 = sb[f"wm2{path}"], sb[f"bm2{path}"]
        mlp = [work.tile([128, NOWN], F32, tag=f"mlp{i}", name=f"mlp{i}") for i in range(2)]
        for m in range(2):
            ms = slice(m * 128, (m + 1) * 128)
            psum = ps_p.tile([128, NOWN], F32, tag="ps_proj", name="ps_proj")
            for j in range(8):
                nc.tensor.matmul(psum, wm2[j][:, ms], h1[j],
                                 start=(j == 0), stop=(j == 7))
            nc.scalar.activation(mlp[m][:], psum[:], AF.Identity,
                                 bias=bm2[m][:], scale=1.0)
        ln1 = [work.tile([128, NOWN], F32, tag=f"ln1_{i}", name=f"ln1_{i}") for i in range(2)]
        layer_norm(mlp, sb[f"gln{path}"], sb[f"bln{path}"], ln1)
        res = [work.tile([128, NOWN], F32, tag=f"res{i}", name=f"res{i}") for i in range(2)]
        for i in range(2):
            nc.vector.tensor_tensor(
                out=res[i][:], in0=ln1[i][:], in1=res_ab[i][:], op=OP.add)
        nrm = [work.tile([128, NOWN], F32, tag=f"nrm{i}", name=f"nrm{i}") for i in range(2)]
        layer_norm(res, sb[gnm], sb[bnm], nrm)
        for i in range(2):
            nc.sync.dma_start(out=odram[i * 128:(i + 1) * 128, :], in_=nrm[i][:])

    ctx.close()
    return nc


_CACHED = {}


def _get_program():
    if "nc" not in _CACHED:
        nc = bacc.Bacc("TRN2", target_bir_lowering=False, num_devices=NCORES)
        nc = _build(nc)
        nc.compile()
        _CACHED["nc"] = nc
    return _CACHED["nc"]


# ----------------------------------------------------------------- wrapper

def kernel(image_embeddings, cnn_embeddings, params):
    img = np.asarray(image_embeddings, np.float32)   # [1, 64, 64, 256]
    cnn = np.asarray(cnn_embeddings, np.float32)     # [1, 256, 64, 64]
    imgT = img.reshape(N, E).T.copy()                # [256, 4096]
    cnnT = cnn.reshape(E, N)                         # [256, 4096]

    shared = _prep_weights(params)
    in_maps = []
    for c in range(NCORES):
        r0 = c * ROWS - HALO
        xi = np.zeros((E, NH), NBF)
        xc = np.zeros((E, NH), NBF)
        vr = np.zeros((1, NH), NBF)
        lo, hi = max(r0, 0), min(r0 + HROWS, HS)
        s0, s1 = (lo - r0) * WS, (hi - r0) * WS
        xi[:, s0:s1] = imgT[:, lo * WS:hi * WS].astype(NBF)
        xc[:, s0:s1] = cnnT[:, lo * WS:hi * WS].astype(NBF)
        vr[:, s0:s1] = 1.0
        m = dict(shared)
        m["ximg"], m["xcnn"], m["vrow"] = xi, xc, vr
        m["rimg"] = np.ascontiguousarray(imgT[:, c * NOWN:(c + 1) * NOWN])
        m["rcnn"] = np.ascontiguousarray(cnnT[:, c * NOWN:(c + 1) * NOWN])
        in_maps.append(m)

    nc = _get_program()
    res = run_bass_kernel_spmd(nc, in_maps, core_ids=list(range(NCORES)))
    sam = np.zeros((HS, WS, E), np.float32)
    cno = np.zeros((E, HS, WS), np.float32)
    for c in range(NCORES):
        o = res.results[c]
        sam[c * ROWS:(c + 1) * ROWS] = (
            o["osam"].T.reshape(ROWS, WS, E))
        cno[:, c * ROWS:(c + 1) * ROWS, :] = (
            o["ocnn"].reshape(E, ROWS, WS))
    return (sam[None], cno[None])


# revision 19
# speedup vs baseline: 1.0078x; 1.0078x over previous
# Trainium2 Bass kernel for nn_DNA_Module_10780367912992 (scrambled-unfold
# sigmoid neighborhood attention, 4 cross branches + fuse/MLP/LN).
#
# Sharding: spatial row-sharding across 8 cores. Core c owns image rows
# [8c, 8c+8) (512 tokens). K/V are computed on a 26-row halo slice per core;
# everything else is per-token. No collectives; host gathers row slices.
#
# On-device layout is channels-on-partitions throughout ("X^T" layout):
#  - linear layers:  Y^T = W @ X^T via PE matmuls (lhsT = W^T, k-chunked)
#  - bias+zero-halo handled by a host-built validity row appended to X^T
#  - NA: for each of the 49 offsets p, the scrambled einsum decomposes into
#      prodK[(h,c),n] = K[(h,c), n+off_p] * Q[(h,(17c+p)%32), n]
#      scores[(h,p'),n] += prodK where p' = (49c+p)//32        (PE scatter)
#      P = sigmoid(scores + rpb)                               (ACT)
#      prodV[(h,k),n] = V[(h,k), n+off_p] * P[(h,(49k+p)//32), n]
#      out[(h,c'),n] += prodV where c' = (49k+p)%32            (PE scatter)
#    The Q-permutation / P-expansion are PE matmuls with 0/1 matrices; the
#    shifted K/V reads are zero-copy APs into a zero-padded [26x82] image.

import numpy as np

import concourse.bass as bass
import concourse.tile as tile
from concourse import bacc, mybir
from concourse.bass_utils import run_bass_kernel_spmd

F32 = mybir.dt.float32
BF16 = mybir.dt.bfloat16
AF = mybir.ActivationFunctionType
OP = mybir.AluOpType

B, HS, WS = 1, 64, 64
E, H, HD = 256, 8, 32
KS, DIL = 7, 3
DS = DC = 256
N = HS * WS
SCALE = HD ** 0.5
NCORES = 8
ROWS = HS // NCORES            # 8 own rows per core
NOWN = ROWS * WS               # 512
HALO = DIL * (KS - 1) // 2     # 9
HROWS = ROWS + 2 * HALO        # 26
NH = HROWS * WS                # 1664
PADX = HALO
WP = WS + 2 * PADX             # 82
IMGF = HROWS * WP              # 2132
OWN0 = HALO * WP + PADX        # own-pixel (0,0) offset in padded image: 747
NP = KS * KS                   # 49
OFFS = [((i - 3) * DIL, (j - 3) * DIL) for i in range(KS) for j in range(KS)]

import ml_dtypes
# stream dtype for heavy tensors; f32 kept for PSUM, LN internals, residuals
DT = BF16
NDT = np.float32
NBF = ml_dtypes.bfloat16


# ---------------------------------------------------------------- host prep

def _sel_matrices():
    ks = np.arange(HD)
    qperm = np.zeros((128, NP * 128), NDT)
    ksel = np.zeros((64, NP * 98), NDT)
    ssel = np.zeros((98, NP * 64), NDT)
    csel = np.zeros((64, NP * 64), NDT)
    for p in range(NP):
        for h in range(4):
            for c in range(HD):
                qperm[h * 32 + (17 * c + p) % 32, p * 128 + h * 32 + c] = 1.0
        for hh in range(2):
            for c in range(HD):
                ksel[hh * 32 + c, p * 98 + hh * NP + (49 * c + p) // 32] = 1.0
                ssel[hh * NP + (49 * c + p) // 32, p * 64 + hh * 32 + c] = 1.0
                csel[hh * 32 + c, p * 64 + hh * 32 + (49 * c + p) % 32] = 1.0
    return qperm, ksel, ssel, csel


def _np(x):
    return np.asarray(x, dtype=np.float32)


def _prep_weights(p):
    """Host-side parameter repack. Returns dict of DRAM input arrays (shared
    across cores)."""
    d = {}

    def lin_T_ones(l, scale=1.0):
        w = _np(l["w"]) * scale        # [dout, din]
        b = _np(l["b"]) * scale        # [dout]
        return np.concatenate([w.T, b[None, :]], 0).astype(NDT)  # [din+1,dout]

    for br, (qn, kn, vn, on) in enumerate(
        [("q_ss", "k_ss", "v_ss", "out_ss"), ("q_sc", "k_sc", "v_sc", "out_sc"),
         ("q_cc", "k_cc", "v_cc", "out_cc"), ("q_cs", "k_cs", "v_cs", "out_cs")]
    ):
        d[f"wq{br}"] = lin_T_ones(p[qn], 1.0 / SCALE).astype(NBF)  # [257,256]
        d[f"wk{br}"] = lin_T_ones(p[kn]).astype(NBF)
        d[f"wv{br}"] = lin_T_ones(p[vn]).astype(NBF)
        d[f"wo{br}"] = _np(p[on]["w"]).T.astype(NBF)    # [256, 256]
        d[f"bo{br}"] = _np(p[on]["b"])[:, None].astype(NDT)  # [256, 1]

    for path, fuse, m1, m2, ln in [
        ("s", "fuse_sam", "mlp_sam_1", "mlp_sam_2", "ln_sam"),
        ("c", "fuse_cnn", "mlp_cnn_1", "mlp_cnn_2", "ln_cnn"),
    ]:
        fw = _np(p[fuse]["w"])                           # [256, 512]
        d[f"wfa{path}"] = fw[:, :256].T.astype(NBF)
        d[f"wfb{path}"] = fw[:, 256:].T.astype(NBF)
        d[f"bf{path}"] = _np(p[fuse]["b"])[:, None].astype(NDT)
        d[f"wm1{path}"] = _np(p[m1]["w"]).T.astype(NBF)  # [256, 1024]
        bm1 = _np(p[m1]["b"])                            # [1024]
        d[f"bm1{path}"] = bm1.reshape(8, 128).T.astype(NDT)  # [128, 8] col/chunk
        d[f"wm2{path}"] = _np(p[m2]["w"]).T.astype(NBF)  # [1024, 256]
        d[f"bm2{path}"] = _np(p[m2]["b"])[:, None].astype(NDT)
        d[f"gln{path}"] = _np(p[ln]["g"])[:, None].astype(NDT)
        d[f"bln{path}"] = _np(p[ln]["b"])[:, None].astype(NDT)

    d["gnms"] = _np(p["norm_sam"]["g"])[:, None].astype(NDT)
    d["bnms"] = _np(p["norm_sam"]["b"])[:, None].astype(NDT)
    d["gnmc"] = _np(p["norm_cnn"]["g"])[:, None].astype(NDT)
    d["bnmc"] = _np(p["norm_cnn"]["b"])[:, None].astype(NDT)

    rpb = _np(p["rpb"]).reshape(H, NP)                   # [8, 49]
    for pp in range(4):
        d[f"rpb{pp}"] = np.concatenate(
            [rpb[2 * pp], rpb[2 * pp + 1]], 0)[:, None].astype(NDT)  # [98,1]

    qperm, ksel, ssel, csel = _sel_matrices()
    d["qperm"], d["ksel"] = qperm.astype(NBF), ksel.astype(NBF)
    d["ssel"], d["csel"] = ssel.astype(NBF), csel.astype(NBF)
    return d


# ---------------------------------------------------------- device program

def _ap3(t_ap, off, d1, d2):
    """[128, (d1 rows, d2 cols)] view of a 2D tile with row stride built in."""
    return bass.AP(tensor=t_ap.tensor, offset=t_ap.offset + off,
                   ap=[t_ap.ap[0], list(d1), list(d2)])


def _build(nc):
    from contextlib import ExitStack
    ctx = ExitStack()
    tc = ctx.enter_context(tile.TileContext(nc))

    din = {}
    def dram_in(name, shape, dt=F32):
        din[name] = nc.dram_tensor(name, shape, dt, kind="ExternalInput")
        return din[name]

    # inputs
    dram_in("ximg", [E, NH], BF16); dram_in("xcnn", [E, NH], BF16)
    dram_in("vrow", [1, NH], BF16)
    dram_in("rimg", [E, NOWN], BF16); dram_in("rcnn", [E, NOWN], BF16)
    for br in range(4):
        dram_in(f"wq{br}", [E + 1, E], BF16); dram_in(f"wk{br}", [E + 1, E], BF16)
        dram_in(f"wv{br}", [E + 1, E], BF16); dram_in(f"wo{br}", [E, E], BF16)
        dram_in(f"bo{br}", [E, 1])
    for path in "sc":
        dram_in(f"wfa{path}", [E, E], BF16); dram_in(f"wfb{path}", [E, E], BF16)
        dram_in(f"bf{path}", [E, 1])
        dram_in(f"wm1{path}", [E, 4 * E], BF16); dram_in(f"bm1{path}", [128, 8])
        dram_in(f"wm2{path}", [4 * E, E], BF16); dram_in(f"bm2{path}", [E, 1])
        dram_in(f"gln{path}", [E, 1]); dram_in(f"bln{path}", [E, 1])
    dram_in("gnms", [E, 1]); dram_in("bnms", [E, 1])
    dram_in("gnmc", [E, 1]); dram_in("bnmc", [E, 1])
    for pp in range(4):
        dram_in(f"rpb{pp}", [98, 1])
    dram_in("qperm", [128, NP * 128], BF16); dram_in("ksel", [64, NP * 98], BF16)
    dram_in("ssel", [98, NP * 64], BF16); dram_in("csel", [64, NP * 64], BF16)

    osam = nc.dram_tensor("osam", [E, NOWN], F32, kind="ExternalOutput")
    ocnn = nc.dram_tensor("ocnn", [E, NOWN], F32, kind="ExternalOutput")

    consts = ctx.enter_context(tc.tile_pool(name="consts", bufs=1))
    imgs = ctx.enter_context(tc.tile_pool(name="imgs", bufs=1))
    work = ctx.enter_context(tc.tile_pool(name="work", bufs=1))
    pp = ctx.enter_context(tc.tile_pool(name="pp", bufs=3))
    acts = ctx.enter_context(tc.tile_pool(name="acts", bufs=1))
    ps_a = ctx.enter_context(tc.tile_pool(name="ps_a", bufs=2, space="PSUM"))
    ps_p = ctx.enter_context(tc.tile_pool(name="ps_p", bufs=1, space="PSUM"))
    ps_sc = ctx.enter_context(tc.tile_pool(name="ps_sc", bufs=1, space="PSUM"))
    ps_v = ctx.enter_context(tc.tile_pool(name="ps_v", bufs=1, space="PSUM"))
    ps_ln = ctx.enter_context(tc.tile_pool(name="ps_ln", bufs=1, space="PSUM"))

    def load(name, shape=None, dt=None):
        dt = dt or din[name].dtype
        t = consts.tile(shape or list(din[name].shape), dt, tag=name, name=name)
        nc.sync.dma_start(out=t, in_=din[name][:])
        return t

    def load2(name, dt=None):
        """Load a >=128-row DRAM tensor as a list of [128, cols] tiles."""
        dt = dt or din[name].dtype
        rows, cols = din[name].shape
        out = []
        for i in range(0, rows, 128):
            t = consts.tile([min(128, rows - i), cols], dt, tag=f"{name}_{i}", name=f"{name}_{i}")
            nc.sync.dma_start(out=t, in_=din[name][i:i + min(128, rows - i), :])
            out.append(t)
        return out

    # resident constants
    sb = {}
    for br in range(4):
        for w in ("wq", "wk", "wv"):
            full = din[f"{w}{br}"]
            a = consts.tile([128, E], DT, tag=f"{w}{br}a", name=f"{w}{br}a")
            b = consts.tile([128, E], DT, tag=f"{w}{br}b", name=f"{w}{br}b")
            r = consts.tile([1, E], DT, tag=f"{w}{br}r", name=f"{w}{br}r")
            nc.sync.dma_start(out=a, in_=full[0:128, :])
            nc.sync.dma_start(out=b, in_=full[128:256, :])
            nc.sync.dma_start(out=r, in_=full[256:257, :])
            sb[f"{w}{br}"] = (a, b, r)
        sb[f"wo{br}"] = load2(f"wo{br}")
        sb[f"bo{br}"] = load2(f"bo{br}")
    for path in "sc":
        for nm in (f"wfa{path}", f"wfb{path}", f"bf{path}", f"wm1{path}",
                   f"wm2{path}", f"bm2{path}", f"gln{path}", f"bln{path}"):
            sb[nm] = load2(nm)
        sb[f"bm1{path}"] = load(f"bm1{path}")
    for nm in ("gnms", "bnms", "gnmc", "bnmc"):
        sb[nm] = load2(nm)
    for nm in ("qperm", "ssel", "rpb0", "rpb1", "rpb2", "rpb3"):
        sb[nm] = load(nm)
    for nm in ("ksel", "csel"):
        cols = din[nm].shape[1]
        t = consts.tile([128, cols], BF16, tag=nm, name=nm)
        nc.sync.dma_start(out=t[0:64, :], in_=din[nm][:])
        nc.sync.dma_start(out=t[64:128, :], in_=din[nm][:])
        sb[nm] = t

    ones128 = consts.tile([128, 1], F32, tag="ones128", name="ones128")
    nc.vector.memset(ones128, 1.0)
    eps1 = consts.tile([1, 1], F32, tag="eps1", name="eps1")
    nc.vector.memset(eps1, 1e-5)

    # activations: x^T tiles (k-chunks) + validity row
    xi_a = acts.tile([128, NH], DT, tag="xi_a", name="xi_a")
    xi_b = acts.tile([128, NH], DT, tag="xi_b", name="xi_b")
    xc_a = acts.tile([128, NH], DT, tag="xc_a", name="xc_a")
    xc_b = acts.tile([128, NH], DT, tag="xc_b", name="xc_b")
    vrow = acts.tile([1, NH], DT, tag="vrow", name="vrow")
    nc.sync.dma_start(out=xi_a, in_=din["ximg"][0:128, :])
    nc.sync.dma_start(out=xi_b, in_=din["ximg"][128:256, :])
    nc.sync.dma_start(out=xc_a, in_=din["xcnn"][0:128, :])
    nc.sync.dma_start(out=xc_b, in_=din["xcnn"][128:256, :])
    nc.sync.dma_start(out=vrow, in_=din["vrow"][:])
    rimg = [acts.tile([128, NOWN], BF16, tag=f"rimg{i}", name=f"rimg{i}") for i in range(2)]
    rcnn = [acts.tile([128, NOWN], BF16, tag=f"rcnn{i}", name=f"rcnn{i}") for i in range(2)]
    for i in range(2):
        nc.sync.dma_start(out=rimg[i], in_=din["rimg"][i * 128:(i + 1) * 128, :])
        nc.sync.dma_start(out=rcnn[i], in_=din["rcnn"][i * 128:(i + 1) * 128, :])

    OC0 = HALO * WS                    # own-column start in halo token space

    def proj(wkey, xab, cols, psum, m0, start=True, stop=True):
        """psum[128, len] += W[m0:m0+128,:] @ x[:, cols] with bias via vrow."""
        a, b, r = sb[wkey]
        xa, xb = xab
        c0, c1 = cols
        nc.tensor.matmul(psum, a[:, m0:m0 + 128], xa[:, c0:c1], start=start, stop=False)
        nc.tensor.matmul(psum, b[:, m0:m0 + 128], xb[:, c0:c1], start=False, stop=False)
        nc.tensor.matmul(psum, r[:, m0:m0 + 128], vrow[:, c0:c1], start=False, stop=stop)

    # ---------------- per-branch compute ----------------
    branch_src = [  # (q-source, kv-source)
        ((xi_a, xi_b), (xi_a, xi_b)), ((xi_a, xi_b), (xc_a, xc_b)),
        ((xc_a, xc_b), (xc_a, xc_b)), ((xc_a, xc_b), (xi_a, xi_b)),
    ]
    outT = {}   # branch -> [2 tiles [128, NOWN]] attention+out-proj result

    ROWCH = [(0, 7), (7, 14), (14, 20), (20, 26)]  # halo row chunks

    for br in range(4):
        qsrc, kvsrc = branch_src[br]

        # K/V padded images per head-group: img[(4h x 32c), 26*82]
        kimg = [imgs.tile([128, IMGF], DT, tag=f"kimg{g}", name=f"kimg{g}") for g in range(2)]
        vimg = [imgs.tile([128, IMGF], DT, tag=f"vimg{g}", name=f"vimg{g}") for g in range(2)]
        kimgO = [imgs.tile([128, IMGF], DT, tag=f"kimgO{g}", name=f"kimgO{g}") for g in range(2)]
        vimgO = [imgs.tile([128, IMGF], DT, tag=f"vimgO{g}", name=f"vimgO{g}") for g in range(2)]
        for g in range(2):
            nc.vector.memset(kimg[g], 0.0)
            nc.vector.memset(vimg[g], 0.0)
        for side, wname, dst in (("k", "wk", kimg), ("v", "wv", vimg)):
            for g in range(2):
                for (r0, r1) in ROWCH:
                    nr = r1 - r0
                    fd = nr * WS
                    psum = ps_p.tile([128, 512], F32, tag="ps_proj", name="ps_proj")
                    proj(f"{wname}{br}", kvsrc, (r0 * WS, r1 * WS),
                         psum[:, 0:fd], g * 128)
                    pap = psum[:]
                    src = bass.AP(tensor=pap.tensor, offset=pap.offset,
                                  ap=[pap.ap[0], [WS, nr], [1, WS]])
                    dstap = _ap3(dst[g][:], r0 * WP + PADX, [WP, nr], [1, WS])
                    nc.vector.tensor_copy(dstap, src)

        for g in range(2):
            nc.vector.tensor_copy(kimgO[g][:, 0:IMGF - 1], kimg[g][:, 1:IMGF])
            nc.vector.tensor_copy(vimgO[g][:, 0:IMGF - 1], vimg[g][:, 1:IMGF])

        # Q per group [128 = 4h x 32c, NOWN]
        q4 = [work.tile([128, NOWN], DT, tag="q4", name="q4") for _ in range(2)]
        for g in range(2):
            psum = ps_p.tile([128, 512], F32, tag="ps_proj", name="ps_proj")
            proj(f"wq{br}", qsrc, (OC0, OC0 + NOWN), psum, g * 128)
            nc.scalar.activation(q4[g][:], psum[:], AF.Identity)

        attn = [work.tile([128, NOWN], DT, tag=f"attn{g}", name=f"attn{g}") for g in range(2)]

        for g in range(2):
            # ---- K loop: scores ----
            sc01 = ps_sc.tile([98, NOWN], F32, tag="sc01", name="sc01")
            sc23 = ps_sc.tile([98, NOWN], F32, tag="sc23", name="sc23")
            for p in range(NP):
                di, dj = OFFS[p]
                off = di * WP + dj
                qp = ps_a.tile([128, NOWN], F32, tag="qp", name="qp")
                nc.tensor.matmul(qp, sb["qperm"][:, p * 128:(p + 1) * 128],
                                 q4[g], start=True, stop=True)
                qps = pp.tile([128, NOWN], DT, tag="qps", name="qps")
                nc.scalar.activation(qps[:], qp[:], AF.Identity)
                prod = pp.tile([128, NOWN], DT, tag="prod", name="prod")
                o0 = OWN0 + off
                src_img, o0 = (kimg[g], o0) if o0 % 2 == 0 else (kimgO[g], o0 - 1)
                in0 = _ap3(src_img[:], o0, [WP, ROWS], [1, WS])
                in1 = _ap3(qps[:], 0, [WS, ROWS], [1, WS])
                o3 = _ap3(prod[:], 0, [WS, ROWS], [1, WS])
                nc.vector.tensor_tensor(out=o3, in0=in0, in1=in1, op=OP.mult)
                nc.tensor.matmul(sc01, sb["ksel"][0:64, p * 98:(p + 1) * 98],
                                 prod[0:64, :], start=(p == 0),
                                 stop=(p == NP - 1))
                nc.tensor.matmul(sc23, sb["ksel"][64:128, p * 98:(p + 1) * 98],
                                 prod[64:128, :], start=(p == 0),
                                 stop=(p == NP - 1))
            # ---- sigmoid(scores + rpb) ----
            p01 = work.tile([98, NOWN], DT, tag="p01", name="p01")
            p23 = work.tile([98, NOWN], DT, tag="p23", name="p23")
            nc.scalar.activation(p01[:], sc01[:], AF.Sigmoid,
                                 bias=sb[f"rpb{2 * g}"][:], scale=1.0)
            nc.scalar.activation(p23[:], sc23[:], AF.Sigmoid,
                                 bias=sb[f"rpb{2 * g + 1}"][:], scale=1.0)
            # ---- V loop ----
            vout0 = ps_v.tile([64, NOWN], F32, tag="vout0", name="vout0")
            vout1 = ps_v.tile([64, NOWN], F32, tag="vout1", name="vout1")
            for p in range(NP):
                di, dj = OFFS[p]
                off = di * WP + dj
                prep = ps_a.tile([128, NOWN], F32, tag="qp", name="prep")
                ssl = sb["ssel"][:, p * 64:(p + 1) * 64]
                nc.tensor.matmul(prep[0:64, :], ssl, p01, start=True, stop=True)
                nc.tensor.matmul(prep[64:128, :], ssl, p23, start=True, stop=True)
                preps = pp.tile([128, NOWN], DT, tag="preps", name="preps")
                nc.scalar.activation(preps[:], prep[:], AF.Identity)
                prodv = pp.tile([128, NOWN], DT, tag="prodv", name="prodv")
                o0 = OWN0 + off
                src_img, o0 = (vimg[g], o0) if o0 % 2 == 0 else (vimgO[g], o0 - 1)
                in0 = _ap3(src_img[:], o0, [WP, ROWS], [1, WS])
                in1 = _ap3(preps[:], 0, [WS, ROWS], [1, WS])
                o3 = _ap3(prodv[:], 0, [WS, ROWS], [1, WS])
                nc.vector.tensor_tensor(out=o3, in0=in0, in1=in1, op=OP.mult)
                nc.tensor.matmul(vout0[:], sb["csel"][0:64, p * 64:(p + 1) * 64],
                                 prodv[0:64, :], start=(p == 0), stop=(p == NP - 1))
                nc.tensor.matmul(vout1[:], sb["csel"][64:128, p * 64:(p + 1) * 64],
                                 prodv[64:128, :], start=(p == 0), stop=(p == NP - 1))
            nc.scalar.activation(attn[g][0:64, :], vout0[:], AF.Identity)
            nc.scalar.activation(attn[g][64:128, :], vout1[:], AF.Identity)

        # ---- out projection [256, NOWN] ----
        oT = [work.tile([128, NOWN], DT, tag=f"oT{br}_{g}", name=f"oT{br}_{g}") for g in range(2)]
        wo, bo = sb[f"wo{br}"], sb[f"bo{br}"]
        for m in range(2):
            psum = ps_p.tile([128, NOWN], F32, tag="ps_proj", name="ps_proj")
            nc.tensor.matmul(psum, wo[0][:, m * 128:(m + 1) * 128], attn[0],
                             start=True, stop=False)
            nc.tensor.matmul(psum, wo[1][:, m * 128:(m + 1) * 128], attn[1],
                             start=False, stop=True)
            nc.scalar.activation(oT[m][:], psum[:], AF.Identity,
                                 bias=bo[m][:], scale=1.0)
        outT[br] = oT

    # ---------------- fuse + MLP + LN per path ----------------
    def layer_norm(xin2, g_ap, b_ap, out2):
        """LN over channel dim (256 = 2 partition chunks) of [2][128, NOWN]."""
        mu = ps_ln.tile([1, NOWN], F32, tag="ln_ps", name="ln_mu")
        nc.tensor.matmul(mu, ones128, xin2[0], start=True, stop=False)
        nc.tensor.matmul(mu, ones128, xin2[1], start=False, stop=True)
        mu_s = work.tile([1, NOWN], F32, tag="ln_mu_s", name="ln_mu_s")
        nc.scalar.activation(mu_s[:], mu[:], AF.Identity, scale=1.0 / E)
        mu_b = work.tile([128, NOWN], F32, tag="ln_mu_b", name="ln_mu_b")
        nc.gpsimd.partition_broadcast(mu_b[:], mu_s[:])
        xc = [work.tile([128, NOWN], F32, tag=f"ln_xc{i}", name=f"ln_xc{i}") for i in range(2)]
        sq = [work.tile([128, NOWN], F32, tag=f"ln_sq{i}", name=f"ln_sq{i}") for i in range(2)]
        for i in range(2):
            nc.vector.tensor_tensor(out=xc[i][:], in0=xin2[i][:], in1=mu_b[:],
                                    op=OP.subtract)
            nc.scalar.activation(sq[i][:], xc[i][:], AF.Square)
        var = ps_ln.tile([1, NOWN], F32, tag="ln_ps", name="ln_var")
        nc.tensor.matmul(var, ones128, sq[0], start=True, stop=False)
        nc.tensor.matmul(var, ones128, sq[1], start=False, stop=True)
        sd = work.tile([1, NOWN], F32, tag="ln_sd", name="ln_sd")
        nc.scalar.activation(sd[:], var[:], AF.Sqrt, bias=eps1[:], scale=1.0 / E)
        rs = work.tile([1, NOWN], F32, tag="ln_rs", name="ln_rs")
        nc.vector.reciprocal(rs[:], sd[:])
        rs_b = work.tile([128, NOWN], F32, tag="ln_rs_b", name="ln_rs_b")
        nc.gpsimd.partition_broadcast(rs_b[:], rs[:])
        for i in range(2):
            xn = work.tile([128, NOWN], F32, tag=f"ln_xn{i}", name=f"ln_xn{i}")
            nc.vector.tensor_tensor(out=xn[:], in0=xc[i][:], in1=rs_b[:],
                                    op=OP.mult)
            nc.scalar.activation(out2[i][:], xn[:], AF.Identity,
                                 scale=g_ap[i][:], bias=b_ap[i][:])

    for path, (ba, bb), res_ab, gnm, bnm, odram in (
        ("s", (0, 1), rimg, "gnms", "bnms", osam),
        ("c", (2, 3), rcnn, "gnmc", "bnmc", ocnn),
    ):
        a0, a1 = outT[ba]
        b0, b1 = outT[bb]
        wfa, wfb = sb[f"wfa{path}"], sb[f"wfb{path}"]
        fuse = [work.tile([128, NOWN], DT, tag=f"fuse{i}", name=f"fuse{i}") for i in range(2)]
        for m in range(2):
            ms = slice(m * 128, (m + 1) * 128)
            psum = ps_p.tile([128, NOWN], F32, tag="ps_proj", name="ps_proj")
            nc.tensor.matmul(psum, wfa[0][:, ms], a0, start=True, stop=False)
            nc.tensor.matmul(psum, wfa[1][:, ms], a1, start=False, stop=False)
            nc.tensor.matmul(psum, wfb[0][:, ms], b0, start=False, stop=False)
            nc.tensor.matmul(psum, wfb[1][:, ms], b1, start=False, stop=True)
            nc.scalar.activation(fuse[m][:], psum[:], AF.Identity,
                                 bias=sb[f"bf{path}"][m][:], scale=1.0)
        wm1, bm1 = sb[f"wm1{path}"], sb[f"bm1{path}"]
        h1 = [work.tile([128, NOWN], DT, tag=f"h1_{j}", name=f"h1_{j}") for j in range(8)]
        for j in range(8):
            ms = slice(j * 128, (j + 1) * 128)
            psum = ps_p.tile([128, NOWN], F32, tag="ps_proj", name="ps_proj")
            nc.tensor.matmul(psum, wm1[0][:, ms], fuse[0], start=True, stop=False)
            nc.tensor.matmul(psum, wm1[1][:, ms], fuse[1], start=False, stop=True)
            nc.scalar.activation(h1[j][:], psum[:], AF.Gelu,
                                 bias=bm1[:, j:j + 1], scale=1.0)
        wm2, bm2 = sb[f"wm2{path}"], sb[f"bm2{path}"]
        mlp = [work.tile([128, NOWN], F32, tag=f"mlp{i}", name=f"mlp{i}") for i in range(2)]
        for m in range(2):
            ms = slice(m * 128, (m + 1) * 128)
            psum = ps_p.tile([128, NOWN], F32, tag="ps_proj", name="ps_proj")
            for j in range(8):
                nc.tensor.matmul(psum, wm2[j][:, ms], h1[j],
                                 start=(j == 0), stop=(j == 7))
            nc.scalar.activation(mlp[m][:], psum[:], AF.Identity,
                                 bias=bm2[m][:], scale=1.0)
        ln1 = [work.tile([128, NOWN], F32, tag=f"ln1_{i}", name=f"ln1_{i}") for i in range(2)]
        layer_norm(mlp, sb[f"gln{path}"], sb[f"bln{path}"], ln1)
        res = [work.tile([128, NOWN], F32, tag=f"res{i}", name=f"res{i}") for i in range(2)]
        for i in range(2):
            nc.vector.tensor_tensor(
                out=res[i][:], in0=ln1[i][:], in1=res_ab[i][:], op=OP.add)
        nrm = [work.tile([128, NOWN], F32, tag=f"nrm{i}", name=f"nrm{i}") for i in range(2)]
        layer_norm(res, sb[gnm], sb[bnm], nrm)
        for i in range(2):
            nc.sync.dma_start(out=odram[i * 128:(i + 1) * 128, :], in_=nrm[i][:])

    ctx.close()
    return nc


_CACHED = {}


def _get_program():
    if "nc" not in _CACHED:
        nc = bacc.Bacc("TRN2", target_bir_lowering=False, num_devices=NCORES)
        nc = _build(nc)
        nc.compile()
        _CACHED["nc"] = nc
    return _CACHED["nc"]


# ----------------------------------------------------------------- wrapper

def kernel(image_embeddings, cnn_embeddings, params):
    img = np.asarray(image_embeddings, np.float32)   # [1, 64, 64, 256]
    cnn = np.asarray(cnn_embeddings, np.float32)     # [1, 256, 64, 64]
    imgT = img.reshape(N, E).T.copy()                # [256, 4096]
    cnnT = cnn.reshape(E, N)                         # [256, 4096]

    shared = _prep_weights(params)
    in_maps = []
    for c in range(NCORES):
        r0 = c * ROWS - HALO
        xi = np.zeros((E, NH), NBF)
        xc = np.zeros((E, NH), NBF)
        vr = np.zeros((1, NH), NBF)
        lo, hi = max(r0, 0), min(r0 + HROWS, HS)
        s0, s1 = (lo - r0) * WS, (hi - r0) * WS
        xi[:, s0:s1] = imgT[:, lo * WS:hi * WS].astype(NBF)
        xc[:, s0:s1] = cnnT[:, lo * WS:hi * WS].astype(NBF)
        vr[:, s0:s1] = 1.0
        m = dict(shared)
        m["ximg"], m["xcnn"], m["vrow"] = xi, xc, vr
        m["rimg"] = imgT[:, c * NOWN:(c + 1) * NOWN].astype(NBF)
        m["rcnn"] = cnnT[:, c * NOWN:(c + 1) * NOWN].astype(NBF)
        in_maps.append(m)

    nc = _get_program()
    res = run_bass_kernel_spmd(nc, in_maps, core_ids=list(range(NCORES)))
    sam = np.zeros((HS, WS, E), np.float32)
    cno = np.zeros((E, HS, WS), np.float32)
    for c in range(NCORES):
        o = res.results[c]
        sam[c * ROWS:(c + 1) * ROWS] = (
            o["osam"].T.reshape(ROWS, WS, E))
        cno[:, c * ROWS:(c + 1) * ROWS, :] = (
            o["ocnn"].reshape(E, ROWS, WS))
    return (sam[None], cno[None])


# revision 25
# speedup vs baseline: 1.0550x; 1.0468x over previous
# Trainium2 Bass kernel for nn_DNA_Module_10780367912992 (scrambled-unfold
# sigmoid neighborhood attention, 4 cross branches + fuse/MLP/LN).
#
# Sharding: spatial row-sharding across 8 cores. Core c owns image rows
# [8c, 8c+8) (512 tokens). K/V are computed on a 26-row halo slice per core;
# everything else is per-token. No collectives; host gathers row slices.
#
# On-device layout is channels-on-partitions throughout ("X^T" layout):
#  - linear layers:  Y^T = W @ X^T via PE matmuls (lhsT = W^T, k-chunked)
#  - bias+zero-halo handled by a host-built validity row appended to X^T
#  - NA: for each of the 49 offsets p, the scrambled einsum decomposes into
#      prodK[(h,c),n] = K[(h,c), n+off_p] * Q[(h,(17c+p)%32), n]
#      scores[(h,p'),n] += prodK where p' = (49c+p)//32        (PE scatter)
#      P = sigmoid(scores + rpb)                               (ACT)
#      prodV[(h,k),n] = V[(h,k), n+off_p] * P[(h,(49k+p)//32), n]
#      out[(h,c'),n] += prodV where c' = (49k+p)%32            (PE scatter)
#    The Q-permutation / P-expansion are PE matmuls with 0/1 matrices; the
#    shifted K/V reads are zero-copy APs into a zero-padded [26x82] image.

import numpy as np

import concourse.bass as bass
import concourse.tile as tile
from concourse import bacc, mybir
from concourse.bass_utils import run_bass_kernel_spmd

F32 = mybir.dt.float32
BF16 = mybir.dt.bfloat16
AF = mybir.ActivationFunctionType
OP = mybir.AluOpType

B, HS, WS = 1, 64, 64
E, H, HD = 256, 8, 32
KS, DIL = 7, 3
DS = DC = 256
N = HS * WS
SCALE = HD ** 0.5
NCORES = 8
ROWS = HS // NCORES            # 8 own rows per core
NOWN = ROWS * WS               # 512
HALO = DIL * (KS - 1) // 2     # 9
HROWS = ROWS + 2 * HALO        # 26
NH = HROWS * WS                # 1664
PADX = HALO
WP = WS + 2 * PADX             # 82
IMGF = HROWS * WP              # 2132
OWN0 = HALO * WP + PADX        # own-pixel (0,0) offset in padded image: 747
NP = KS * KS                   # 49
OFFS = [((i - 3) * DIL, (j - 3) * DIL) for i in range(KS) for j in range(KS)]

import ml_dtypes
# stream dtype for heavy tensors; f32 kept for PSUM, LN internals, residuals
DT = BF16
NDT = np.float32
NBF = ml_dtypes.bfloat16


# ---------------------------------------------------------------- host prep

def _sel_matrices():
    ks = np.arange(HD)
    qperm = np.zeros((128, NP * 128), NDT)
    ksel = np.zeros((64, NP * 98), NDT)
    ssel = np.zeros((98, NP * 64), NDT)
    csel = np.zeros((64, NP * 64), NDT)
    for p in range(NP):
        for h in range(4):
            for c in range(HD):
                qperm[h * 32 + (17 * c + p) % 32, p * 128 + h * 32 + c] = 1.0
        for hh in range(2):
            for c in range(HD):
                ksel[hh * 32 + c, p * 98 + hh * NP + (49 * c + p) // 32] = 1.0
                ssel[hh * NP + (49 * c + p) // 32, p * 64 + hh * 32 + c] = 1.0
                csel[hh * 32 + c, p * 64 + hh * 32 + (49 * c + p) % 32] = 1.0
    return qperm, ksel, ssel, csel


def _np(x):
    return np.asarray(x, dtype=np.float32)


def _prep_weights(p):
    """Host-side parameter repack. Returns dict of DRAM input arrays (shared
    across cores)."""
    d = {}

    def lin_T_ones(l, scale=1.0):
        w = _np(l["w"]) * scale        # [dout, din]
        b = _np(l["b"]) * scale        # [dout]
        return np.concatenate([w.T, b[None, :]], 0).astype(NDT)  # [din+1,dout]

    for br, (qn, kn, vn, on) in enumerate(
        [("q_ss", "k_ss", "v_ss", "out_ss"), ("q_sc", "k_sc", "v_sc", "out_sc"),
         ("q_cc", "k_cc", "v_cc", "out_cc"), ("q_cs", "k_cs", "v_cs", "out_cs")]
    ):
        d[f"wq{br}"] = lin_T_ones(p[qn], 1.0 / SCALE).astype(NBF)  # [257,256]
        d[f"wk{br}"] = lin_T_ones(p[kn]).astype(NBF)
        d[f"wv{br}"] = lin_T_ones(p[vn]).astype(NBF)
        d[f"wo{br}"] = _np(p[on]["w"]).T.astype(NBF)    # [256, 256]
        d[f"bo{br}"] = _np(p[on]["b"])[:, None].astype(NDT)  # [256, 1]

    for path, fuse, m1, m2, ln in [
        ("s", "fuse_sam", "mlp_sam_1", "mlp_sam_2", "ln_sam"),
        ("c", "fuse_cnn", "mlp_cnn_1", "mlp_cnn_2", "ln_cnn"),
    ]:
        fw = _np(p[fuse]["w"])                           # [256, 512]
        d[f"wfa{path}"] = fw[:, :256].T.astype(NBF)
        d[f"wfb{path}"] = fw[:, 256:].T.astype(NBF)
        d[f"bf{path}"] = _np(p[fuse]["b"])[:, None].astype(NDT)
        d[f"wm1{path}"] = _np(p[m1]["w"]).T.astype(NBF)  # [256, 1024]
        bm1 = _np(p[m1]["b"])                            # [1024]
        d[f"bm1{path}"] = bm1.reshape(8, 128).T.astype(NDT)  # [128, 8] col/chunk
        d[f"wm2{path}"] = _np(p[m2]["w"]).T.astype(NBF)  # [1024, 256]
        d[f"bm2{path}"] = _np(p[m2]["b"])[:, None].astype(NDT)
        d[f"gln{path}"] = _np(p[ln]["g"])[:, None].astype(NDT)
        d[f"bln{path}"] = _np(p[ln]["b"])[:, None].astype(NDT)

    d["gnms"] = _np(p["norm_sam"]["g"])[:, None].astype(NDT)
    d["bnms"] = _np(p["norm_sam"]["b"])[:, None].astype(NDT)
    d["gnmc"] = _np(p["norm_cnn"]["g"])[:, None].astype(NDT)
    d["bnmc"] = _np(p["norm_cnn"]["b"])[:, None].astype(NDT)

    rpb = _np(p["rpb"]).reshape(H, NP)                   # [8, 49]
    for pp in range(4):
        d[f"rpb{pp}"] = np.concatenate(
            [rpb[2 * pp], rpb[2 * pp + 1]], 0)[:, None].astype(NDT)  # [98,1]

    qperm, ksel, ssel, csel = _sel_matrices()
    d["qperm"], d["ksel"] = qperm.astype(NBF), ksel.astype(NBF)
    d["ssel"], d["csel"] = ssel.astype(NBF), csel.astype(NBF)
    return d


# ---------------------------------------------------------- device program

def _ap3(t_ap, off, d1, d2):
    """[128, (d1 rows, d2 cols)] view of a 2D tile with row stride built in."""
    return bass.AP(tensor=t_ap.tensor, offset=t_ap.offset + off,
                   ap=[t_ap.ap[0], list(d1), list(d2)])


def _build(nc):
    from contextlib import ExitStack
    ctx = ExitStack()
    tc = ctx.enter_context(tile.TileContext(nc))

    din = {}
    def dram_in(name, shape, dt=F32):
        din[name] = nc.dram_tensor(name, shape, dt, kind="ExternalInput")
        return din[name]

    # inputs
    dram_in("ximg", [E, NH], BF16); dram_in("xcnn", [E, NH], BF16)
    dram_in("vrow", [1, NH], BF16)
    dram_in("rimg", [E, NOWN], BF16); dram_in("rcnn", [E, NOWN], BF16)
    for br in range(4):
        dram_in(f"wq{br}", [E + 1, E], BF16); dram_in(f"wk{br}", [E + 1, E], BF16)
        dram_in(f"wv{br}", [E + 1, E], BF16); dram_in(f"wo{br}", [E, E], BF16)
        dram_in(f"bo{br}", [E, 1])
    for path in "sc":
        dram_in(f"wfa{path}", [E, E], BF16); dram_in(f"wfb{path}", [E, E], BF16)
        dram_in(f"bf{path}", [E, 1])
        dram_in(f"wm1{path}", [E, 4 * E], BF16); dram_in(f"bm1{path}", [128, 8])
        dram_in(f"wm2{path}", [4 * E, E], BF16); dram_in(f"bm2{path}", [E, 1])
        dram_in(f"gln{path}", [E, 1]); dram_in(f"bln{path}", [E, 1])
    dram_in("gnms", [E, 1]); dram_in("bnms", [E, 1])
    dram_in("gnmc", [E, 1]); dram_in("bnmc", [E, 1])
    for pp in range(4):
        dram_in(f"rpb{pp}", [98, 1])
    dram_in("qperm", [128, NP * 128], BF16); dram_in("ksel", [64, NP * 98], BF16)
    dram_in("ssel", [98, NP * 64], BF16); dram_in("csel", [64, NP * 64], BF16)

    osam = nc.dram_tensor("osam", [E, NOWN], F32, kind="ExternalOutput")
    ocnn = nc.dram_tensor("ocnn", [E, NOWN], F32, kind="ExternalOutput")

    consts = ctx.enter_context(tc.tile_pool(name="consts", bufs=1))
    imgs = ctx.enter_context(tc.tile_pool(name="imgs", bufs=1))
    work = ctx.enter_context(tc.tile_pool(name="work", bufs=1))
    pp = ctx.enter_context(tc.tile_pool(name="pp", bufs=2))
    acts = ctx.enter_context(tc.tile_pool(name="acts", bufs=1))
    ps_a = ctx.enter_context(tc.tile_pool(name="ps_a", bufs=2, space="PSUM"))
    ps_p = ps_a
    ps_acc = ctx.enter_context(tc.tile_pool(name="ps_acc", bufs=1, space="PSUM"))
    ps_ln = ps_a

    def load(name, shape=None, dt=None):
        dt = dt or din[name].dtype
        t = consts.tile(shape or list(din[name].shape), dt, tag=name, name=name)
        nc.sync.dma_start(out=t, in_=din[name][:])
        return t

    def load2(name, dt=None):
        """Load a >=128-row DRAM tensor as a list of [128, cols] tiles."""
        dt = dt or din[name].dtype
        rows, cols = din[name].shape
        out = []
        for i in range(0, rows, 128):
            t = consts.tile([min(128, rows - i), cols], dt, tag=f"{name}_{i}", name=f"{name}_{i}")
            nc.sync.dma_start(out=t, in_=din[name][i:i + min(128, rows - i), :])
            out.append(t)
        return out

    # resident constants
    sb = {}
    for br in range(4):
        for w in ("wq", "wk", "wv"):
            full = din[f"{w}{br}"]
            a = consts.tile([128, E], DT, tag=f"{w}{br}a", name=f"{w}{br}a")
            b = consts.tile([128, E], DT, tag=f"{w}{br}b", name=f"{w}{br}b")
            r = consts.tile([1, E], DT, tag=f"{w}{br}r", name=f"{w}{br}r")
            nc.sync.dma_start(out=a, in_=full[0:128, :])
            nc.sync.dma_start(out=b, in_=full[128:256, :])
            nc.sync.dma_start(out=r, in_=full[256:257, :])
            sb[f"{w}{br}"] = (a, b, r)
        sb[f"wo{br}"] = load2(f"wo{br}")
        sb[f"bo{br}"] = load2(f"bo{br}")
    for path in "sc":
        for nm in (f"wfa{path}", f"wfb{path}", f"bf{path}", f"wm1{path}",
                   f"wm2{path}", f"bm2{path}", f"gln{path}", f"bln{path}"):
            sb[nm] = load2(nm)
        sb[f"bm1{path}"] = load(f"bm1{path}")
    for nm in ("gnms", "bnms", "gnmc", "bnmc"):
        sb[nm] = load2(nm)
    for nm in ("qperm", "ssel", "rpb0", "rpb1", "rpb2", "rpb3"):
        sb[nm] = load(nm)
    for nm in ("ksel", "csel"):
        cols = din[nm].shape[1]
        t = consts.tile([128, cols], BF16, tag=nm, name=nm)
        nc.sync.dma_start(out=t[0:64, :], in_=din[nm][:])
        nc.sync.dma_start(out=t[64:128, :], in_=din[nm][:])
        sb[nm] = t

    ones128 = consts.tile([128, 1], F32, tag="ones128", name="ones128")
    nc.vector.memset(ones128, 1.0)
    eps1 = consts.tile([1, 1], F32, tag="eps1", name="eps1")
    nc.vector.memset(eps1, 1e-5)

    # activations: x^T tiles (k-chunks) + validity row
    xi_a = acts.tile([128, NH], DT, tag="xi_a", name="xi_a")
    xi_b = acts.tile([128, NH], DT, tag="xi_b", name="xi_b")
    xc_a = acts.tile([128, NH], DT, tag="xc_a", name="xc_a")
    xc_b = acts.tile([128, NH], DT, tag="xc_b", name="xc_b")
    vrow = acts.tile([1, NH], DT, tag="vrow", name="vrow")
    nc.sync.dma_start(out=xi_a, in_=din["ximg"][0:128, :])
    nc.sync.dma_start(out=xi_b, in_=din["ximg"][128:256, :])
    nc.sync.dma_start(out=xc_a, in_=din["xcnn"][0:128, :])
    nc.sync.dma_start(out=xc_b, in_=din["xcnn"][128:256, :])
    nc.sync.dma_start(out=vrow, in_=din["vrow"][:])
    rimg = [acts.tile([128, NOWN], BF16, tag=f"rimg{i}", name=f"rimg{i}") for i in range(2)]
    rcnn = [acts.tile([128, NOWN], BF16, tag=f"rcnn{i}", name=f"rcnn{i}") for i in range(2)]
    for i in range(2):
        nc.sync.dma_start(out=rimg[i], in_=din["rimg"][i * 128:(i + 1) * 128, :])
        nc.sync.dma_start(out=rcnn[i], in_=din["rcnn"][i * 128:(i + 1) * 128, :])

    OC0 = HALO * WS                    # own-column start in halo token space

    def proj(wkey, xab, cols, psum, m0, start=True, stop=True):
        """psum[128, len] += W[m0:m0+128,:] @ x[:, cols] with bias via vrow."""
        a, b, r = sb[wkey]
        xa, xb = xab
        c0, c1 = cols
        nc.tensor.matmul(psum, a[:, m0:m0 + 128], xa[:, c0:c1], start=start, stop=False)
        nc.tensor.matmul(psum, b[:, m0:m0 + 128], xb[:, c0:c1], start=False, stop=False)
        nc.tensor.matmul(psum, r[:, m0:m0 + 128], vrow[:, c0:c1], start=False, stop=stop)

    # ---------------- per-branch compute ----------------
    branch_src = [  # (q-source, kv-source)
        ((xi_a, xi_b), (xi_a, xi_b)), ((xi_a, xi_b), (xc_a, xc_b)),
        ((xc_a, xc_b), (xc_a, xc_b)), ((xc_a, xc_b), (xi_a, xi_b)),
    ]
    outT = {}   # branch -> [2 tiles [128, NOWN]] attention+out-proj result

    ROWCH = [(0, 7), (7, 14), (14, 20), (20, 26)]  # halo row chunks

    for br in range(4):
        qsrc, kvsrc = branch_src[br]

        # K/V padded images per head-group: img[(4h x 32c), 26*82]
        kimg = [imgs.tile([128, IMGF], DT, tag=f"kimg{g}", name=f"kimg{g}") for g in range(2)]
        vimg = [imgs.tile([128, IMGF], DT, tag=f"vimg{g}", name=f"vimg{g}") for g in range(2)]
        kimgO = [imgs.tile([128, IMGF], DT, tag=f"kimgO{g}", name=f"kimgO{g}") for g in range(2)]
        vimgO = [imgs.tile([128, IMGF], DT, tag=f"vimgO{g}", name=f"vimgO{g}") for g in range(2)]
        for g in range(2):
            nc.vector.memset(kimg[g], 0.0)
            nc.vector.memset(vimg[g], 0.0)
        for side, wname, dst in (("k", "wk", kimg), ("v", "wv", vimg)):
            for g in range(2):
                for (r0, r1) in ROWCH:
                    nr = r1 - r0
                    fd = nr * WS
                    psum = ps_p.tile([128, 512], F32, tag="qp", name="ps_proj")
                    proj(f"{wname}{br}", kvsrc, (r0 * WS, r1 * WS),
                         psum[:, 0:fd], g * 128)
                    pap = psum[:]
                    src = bass.AP(tensor=pap.tensor, offset=pap.offset,
                                  ap=[pap.ap[0], [WS, nr], [1, WS]])
                    dstap = _ap3(dst[g][:], r0 * WP + PADX, [WP, nr], [1, WS])
                    nc.vector.tensor_copy(dstap, src)

        for g in range(2):
            nc.vector.tensor_copy(kimgO[g][:, 0:IMGF - 1], kimg[g][:, 1:IMGF])
            nc.vector.tensor_copy(vimgO[g][:, 0:IMGF - 1], vimg[g][:, 1:IMGF])

        # Q both groups fused: [128 = 4h x 32c, 2*NOWN] (g0 cols, g1 cols)
        q4f = work.tile([128, 2 * NOWN], DT, tag="q4f", name="q4f")
        for g in range(2):
            psum = ps_p.tile([128, 512], F32, tag="qp", name="ps_proj")
            proj(f"wq{br}", qsrc, (OC0, OC0 + NOWN), psum, g * 128)
            nc.scalar.activation(q4f[:, g * NOWN:(g + 1) * NOWN], psum[:],
                                 AF.Identity)

        attn = [work.tile([128, NOWN], DT, tag=f"attn{g}", name=f"attn{g}") for g in range(2)]

        # ---- K loop (both groups per p) ----
        sc = [[ps_acc.tile([98, NOWN], F32, tag=f"acc{g}{hh}", name=f"sc{g}{hh}")
               for hh in range(2)] for g in range(2)]
        for p in range(NP):
            di, dj = OFFS[p]
            off = di * WP + dj
            qp = ps_a.tile([128, 2 * NOWN], F32, tag="qp", name="qp")
            for gg in range(2):
                cs = slice(gg * NOWN, (gg + 1) * NOWN)
                nc.tensor.matmul(qp[:, cs], sb["qperm"][:, p * 128:(p + 1) * 128],
                                 q4f[:, cs], start=True, stop=True)
            qps = pp.tile([128, 2 * NOWN], DT, tag="qps", name="qps")
            nc.scalar.activation(qps[:], qp[:], AF.Identity)
            prod = pp.tile([128, 2 * NOWN], DT, tag="prod", name="prod")
            o0 = OWN0 + off
            for g in range(2):
                src_img, oo = (kimg[g], o0) if o0 % 2 == 0 else (kimgO[g], o0 - 1)
                in0 = _ap3(src_img[:], oo, [WP, ROWS], [1, WS])
                in1 = _ap3(qps[:], g * NOWN, [WS, ROWS], [1, WS])
                o3 = _ap3(prod[:], g * NOWN, [WS, ROWS], [1, WS])
                nc.vector.tensor_tensor(out=o3, in0=in0, in1=in1, op=OP.mult)
                gs = slice(g * NOWN, (g + 1) * NOWN)
                nc.tensor.matmul(sc[g][0], sb["ksel"][0:64, p * 98:(p + 1) * 98],
                                 prod[0:64, gs], start=(p == 0),
                                 stop=(p == NP - 1))
                nc.tensor.matmul(sc[g][1], sb["ksel"][64:128, p * 98:(p + 1) * 98],
                                 prod[64:128, gs], start=(p == 0),
                                 stop=(p == NP - 1))
        # ---- sigmoid(scores + rpb): P tiles fused over g in cols ----
        pf = [work.tile([98, 2 * NOWN], DT, tag=f"pf{hh}", name=f"pf{hh}")
              for hh in range(2)]
        for g in range(2):
            for hh in range(2):
                nc.scalar.activation(pf[hh][:, g * NOWN:(g + 1) * NOWN],
                                     sc[g][hh][:], AF.Sigmoid,
                                     bias=sb[f"rpb{2 * g + hh}"][:], scale=1.0)
        # ---- V loop (both groups per p) ----
        vout = [[ps_acc.tile([64, NOWN], F32, tag=f"acc{g}{hh}", name=f"vout{g}{hh}")
                 for hh in range(2)] for g in range(2)]
        for p in range(NP):
            di, dj = OFFS[p]
            off = di * WP + dj
            prep = ps_a.tile([128, 2 * NOWN], F32, tag="qp", name="prep")
            ssl = sb["ssel"][:, p * 64:(p + 1) * 64]
            for gg in range(2):
                cs = slice(gg * NOWN, (gg + 1) * NOWN)
                nc.tensor.matmul(prep[0:64, cs], ssl, pf[0][:, cs], start=True, stop=True)
                nc.tensor.matmul(prep[64:128, cs], ssl, pf[1][:, cs], start=True, stop=True)
            preps = pp.tile([128, 2 * NOWN], DT, tag="preps", name="preps")
            nc.scalar.activation(preps[:], prep[:], AF.Identity)
            prodv = pp.tile([128, 2 * NOWN], DT, tag="prodv", name="prodv")
            o0 = OWN0 + off
            for g in range(2):
                src_img, oo = (vimg[g], o0) if o0 % 2 == 0 else (vimgO[g], o0 - 1)
                in0 = _ap3(src_img[:], oo, [WP, ROWS], [1, WS])
                in1 = _ap3(preps[:], g * NOWN, [WS, ROWS], [1, WS])
                o3 = _ap3(prodv[:], g * NOWN, [WS, ROWS], [1, WS])
                nc.vector.tensor_tensor(out=o3, in0=in0, in1=in1, op=OP.mult)
                gs = slice(g * NOWN, (g + 1) * NOWN)
                nc.tensor.matmul(vout[g][0][:], sb["csel"][0:64, p * 64:(p + 1) * 64],
                                 prodv[0:64, gs], start=(p == 0), stop=(p == NP - 1))
                nc.tensor.matmul(vout[g][1][:], sb["csel"][64:128, p * 64:(p + 1) * 64],
                                 prodv[64:128, gs], start=(p == 0), stop=(p == NP - 1))
        for g in range(2):
            nc.scalar.activation(attn[g][0:64, :], vout[g][0][:], AF.Identity)
            nc.scalar.activation(attn[g][64:128, :], vout[g][1][:], AF.Identity)

        # ---- out projection [256, NOWN] ----
        oT = [work.tile([128, NOWN], DT, tag=f"oT{br}_{g}", name=f"oT{br}_{g}") for g in range(2)]
        wo, bo = sb[f"wo{br}"], sb[f"bo{br}"]
        for m in range(2):
            psum = ps_p.tile([128, NOWN], F32, tag="qp", name="ps_proj")
            nc.tensor.matmul(psum, wo[0][:, m * 128:(m + 1) * 128], attn[0],
                             start=True, stop=False)
            nc.tensor.matmul(psum, wo[1][:, m * 128:(m + 1) * 128], attn[1],
                             start=False, stop=True)
            nc.scalar.activation(oT[m][:], psum[:], AF.Identity,
                                 bias=bo[m][:], scale=1.0)
        outT[br] = oT

    # ---------------- fuse + MLP + LN per path ----------------
    def layer_norm(xin2, g_ap, b_ap, out2):
        """LN over channel dim (256 = 2 partition chunks) of [2][128, NOWN]."""
        mu = ps_ln.tile([1, NOWN], F32, tag="qp", name="ln_mu")
        nc.tensor.matmul(mu, ones128, xin2[0], start=True, stop=False)
        nc.tensor.matmul(mu, ones128, xin2[1], start=False, stop=True)
        mu_s = work.tile([1, NOWN], F32, tag="ln_mu_s", name="ln_mu_s")
        nc.scalar.activation(mu_s[:], mu[:], AF.Identity, scale=1.0 / E)
        mu_b = work.tile([128, NOWN], F32, tag="ln_mu_b", name="ln_mu_b")
        nc.gpsimd.partition_broadcast(mu_b[:], mu_s[:])
        xc = [work.tile([128, NOWN], F32, tag=f"ln_xc{i}", name=f"ln_xc{i}") for i in range(2)]
        sq = [work.tile([128, NOWN], F32, tag=("ln_mu_b" if i == 0 else "ln_sq1"), name=f"ln_sq{i}") for i in range(2)]
        for i in range(2):
            nc.vector.tensor_tensor(out=xc[i][:], in0=xin2[i][:], in1=mu_b[:],
                                    op=OP.subtract)
            nc.scalar.activation(sq[i][:], xc[i][:], AF.Square)
        var = ps_ln.tile([1, NOWN], F32, tag="qp", name="ln_var")
        nc.tensor.matmul(var, ones128, sq[0], start=True, stop=False)
        nc.tensor.matmul(var, ones128, sq[1], start=False, stop=True)
        sd = work.tile([1, NOWN], F32, tag="ln_sd", name="ln_sd")
        nc.scalar.activation(sd[:], var[:], AF.Sqrt, bias=eps1[:], scale=1.0 / E)
        rs = work.tile([1, NOWN], F32, tag="ln_rs", name="ln_rs")
        nc.vector.reciprocal(rs[:], sd[:])
        rs_b = work.tile([128, NOWN], F32, tag="ln_sq1", name="ln_rs_b")
        nc.gpsimd.partition_broadcast(rs_b[:], rs[:])
        for i in range(2):
            xn = work.tile([128, NOWN], F32, tag=f"ln_xn{i}", name=f"ln_xn{i}")
            nc.vector.tensor_tensor(out=xn[:], in0=xc[i][:], in1=rs_b[:],
                                    op=OP.mult)
            nc.scalar.activation(out2[i][:], xn[:], AF.Identity,
                                 scale=g_ap[i][:], bias=b_ap[i][:])

    for path, (ba, bb), res_ab, gnm, bnm, odram in (
        ("s", (0, 1), rimg, "gnms", "bnms", osam),
        ("c", (2, 3), rcnn, "gnmc", "bnmc", ocnn),
    ):
        a0, a1 = outT[ba]
        b0, b1 = outT[bb]
        wfa, wfb = sb[f"wfa{path}"], sb[f"wfb{path}"]
        fuse = [work.tile([128, NOWN], DT, tag=f"fuse{i}", name=f"fuse{i}") for i in range(2)]
        for m in range(2):
            ms = slice(m * 128, (m + 1) * 128)
            psum = ps_p.tile([128, NOWN], F32, tag="qp", name="ps_proj")
            nc.tensor.matmul(psum, wfa[0][:, ms], a0, start=True, stop=False)
            nc.tensor.matmul(psum, wfa[1][:, ms], a1, start=False, stop=False)
            nc.tensor.matmul(psum, wfb[0][:, ms], b0, start=False, stop=False)
            nc.tensor.matmul(psum, wfb[1][:, ms], b1, start=False, stop=True)
            nc.scalar.activation(fuse[m][:], psum[:], AF.Identity,
                                 bias=sb[f"bf{path}"][m][:], scale=1.0)
        wm1, bm1 = sb[f"wm1{path}"], sb[f"bm1{path}"]
        h1 = [work.tile([128, NOWN], DT, tag=f"h1_{j}", name=f"h1_{j}") for j in range(8)]
        for j in range(8):
            ms = slice(j * 128, (j + 1) * 128)
            psum = ps_p.tile([128, NOWN], F32, tag="qp", name="ps_proj")
            nc.tensor.matmul(psum, wm1[0][:, ms], fuse[0], start=True, stop=False)
            nc.tensor.matmul(psum, wm1[1][:, ms], fuse[1], start=False, stop=True)
            nc.scalar.activation(h1[j][:], psum[:], AF.Gelu,
                                 bias=bm1[:, j:j + 1], scale=1.0)
        wm2, bm2 = sb[f"wm2{path}"], sb[f"bm2{path}"]
        mlp = [work.tile([128, NOWN], F32, tag=f"mlp{i}", name=f"mlp{i}") for i in range(2)]
        for m in range(2):
            ms = slice(m * 128, (m + 1) * 128)
            psum = ps_p.tile([128, NOWN], F32, tag="qp", name="ps_proj")
            for j in range(8):
                nc.tensor.matmul(psum, wm2[j][:, ms], h1[j],
                                 start=(j == 0), stop=(j == 7))
            nc.scalar.activation(mlp[m][:], psum[:], AF.Identity,
                                 bias=bm2[m][:], scale=1.0)
        ln1 = [work.tile([128, NOWN], F32, tag=f"ln1_{i}", name=f"ln1_{i}") for i in range(2)]
        layer_norm(mlp, sb[f"gln{path}"], sb[f"bln{path}"], ln1)
        res = [work.tile([128, NOWN], F32, tag=f"res{i}", name=f"res{i}") for i in range(2)]
        for i in range(2):
            nc.vector.tensor_tensor(
                out=res[i][:], in0=ln1[i][:], in1=res_ab[i][:], op=OP.add)
        nrm = [work.tile([128, NOWN], F32, tag=f"nrm{i}", name=f"nrm{i}") for i in range(2)]
        layer_norm(res, sb[gnm], sb[bnm], nrm)
        for i in range(2):
            nc.sync.dma_start(out=odram[i * 128:(i + 1) * 128, :], in_=nrm[i][:])

    ctx.close()
    return nc


_CACHED = {}


def _get_program():
    if "nc" not in _CACHED:
        nc = bacc.Bacc("TRN2", target_bir_lowering=False, num_devices=NCORES)
        nc = _build(nc)
        nc.compile()
        _CACHED["nc"] = nc
    return _CACHED["nc"]


# ----------------------------------------------------------------- wrapper

def kernel(image_embeddings, cnn_embeddings, params):
    img = np.asarray(image_embeddings, np.float32)   # [1, 64, 64, 256]
    cnn = np.asarray(cnn_embeddings, np.float32)     # [1, 256, 64, 64]
    imgT = img.reshape(N, E).T.copy()                # [256, 4096]
    cnnT = cnn.reshape(E, N)                         # [256, 4096]

    shared = _prep_weights(params)
    in_maps = []
    for c in range(NCORES):
        r0 = c * ROWS - HALO
        xi = np.zeros((E, NH), NBF)
        xc = np.zeros((E, NH), NBF)
        vr = np.zeros((1, NH), NBF)
        lo, hi = max(r0, 0), min(r0 + HROWS, HS)
        s0, s1 = (lo - r0) * WS, (hi - r0) * WS
        xi[:, s0:s1] = imgT[:, lo * WS:hi * WS].astype(NBF)
        xc[:, s0:s1] = cnnT[:, lo * WS:hi * WS].astype(NBF)
        vr[:, s0:s1] = 1.0
        m = dict(shared)
        m["ximg"], m["xcnn"], m["vrow"] = xi, xc, vr
        m["rimg"] = imgT[:, c * NOWN:(c + 1) * NOWN].astype(NBF)
        m["rcnn"] = cnnT[:, c * NOWN:(c + 1) * NOWN].astype(NBF)
        in_maps.append(m)

    nc = _get_program()
    res = run_bass_kernel_spmd(nc, in_maps, core_ids=list(range(NCORES)))
    sam = np.zeros((HS, WS, E), np.float32)
    cno = np.zeros((E, HS, WS), np.float32)
    for c in range(NCORES):
        o = res.results[c]
        sam[c * ROWS:(c + 1) * ROWS] = (
            o["osam"].T.reshape(ROWS, WS, E))
        cno[:, c * ROWS:(c + 1) * ROWS, :] = (
            o["ocnn"].reshape(E, ROWS, WS))
    return (sam[None], cno[None])
